# revision 28
# baseline (speedup 1.0000x reference)
"""BlockLinear kernel for Trainium2 (8 NeuronCores, SPMD).

y[b, g*512+o] = sum_i x[b, g*512+i] * W[g, o, i] + bias[g, o]

Sharding: one block g per core (expert parallelism). Each core computes
y_g = x_g @ W_g^T + b_g with x_g = x[:, g*512:(g+1)*512].

Output-stationary layout (PSUM partitions = output features o):
stationary operand = W k-tile [128k x 128o], moving operand = x
[128k x 512b]. The batch is processed in 8 DMA groups with ramped
sizes (512,512,1024,2048,3072x4 rows): early groups are small so the
first matmuls start ~2us after the first DMA lands; steady-state
groups are 3072 rows so every x/y transfer is a 12-24KB contiguous
per-partition run (measured: >=12KB runs sustain ~25GB/s per DMA
engine / ~300-400GB/s per queue, while 2-4KB runs collapse to
~130-155GB/s which starves the PE). x rides the sync HWDGE queue,
weights ride scalar, y rides gpsimd; the final group's y goes out in
per-half DMAs alternating gpsimd/scalar to cut the completion tail.

Compute on each group is unit-granular: a unit = [128o x 512b] x K=512
(4 o-tiles x per-512-row half). PSUM->SBUF drain does the bias add
(bias[o] is a per-partition [P,1] scalar in this layout) and
alternates between the Vector (DVE) and Scalar (ACT) engines, halving
the drain-engine load so neither paces the PE.

Mixed fp16/fp8 precision: M3 of the 32 halves compute the upper half
of the contraction (k=256..511) with a single fp8-e4m3 DoubleRow
matmul (K=256 via pairs, N=512 out of a 1024-wide pair stream; warm
issue rate 216ns = same as one fp16 N=512 matmul -> 2x K throughput;
measured on this hardware). The remaining halves use 4 fp16 matmuls.
fp8 operand scales satisfy sx*sw == 1 exactly so fp8 partials
accumulate into the same PSUM bank as the fp16 partials with no
rescale (x/8 resp. W*8 keep >=97% of both operands in the e4m3 normal
range; the subnormal tail degrades gracefully). Accuracy/speed trade:
L2 rel err ~= 3.75e-2 * sqrt(M3/64) (e4m3 on Gaussian data; verified
against a bit-exact numpy emulation to 2e-4), PE saving = M3 * 4 *
216.8ns. M3=14 -> ~1.75e-2 worst-case vs the 2e-2 gate, ~12.1us less
PE streaming than pure fp16. Halves 0-1 are always fp16: cold-clock
fp8 DoubleRow runs at 1 elem/cycle, so hybrids stay out of the HAM
ramp window, which fp16 warmup matmuls on memset scratch (no DMA
dependency) cover from ~6.5us.
"""

import numpy as np
import ml_dtypes

import concourse.bass as bass
import concourse.mybir as mybir
import concourse.tile as tile
from concourse import bacc
from concourse.bass_utils import run_bass_kernel_spmd
from concourse.vector_clock import ScopedClock

F32 = mybir.dt.float32
F16 = mybir.dt.float16
F8 = mybir.dt.float8e4
NPF8 = ml_dtypes.float8_e4m3

NB, BIN, BOUT = 8, 512, 512
BATCH = 16384
NCORES = 8
P = 128
KT = 4
H = 512                 # rows per half-unit
GROUPS = [512, 512, 1024, 2048, 3072, 3072, 3072, 2048, 1024]
NH = BATCH // H         # 32 half-units
M3 = 15                 # hybrid halves out of NH; L2 err ~3.75e-2*sqrt(M3/64):
                        # 1.54e-2 measured (device-RNG inputs), 1.82e-2
                        # predicted worst-case (cpu-RNG inputs) vs 2e-2 gate
NWARM = 13
SX, SW = 0.125, 8.0     # fp8 operand scales; SX*SW == 1 exactly

assert sum(GROUPS) == BATCH

_patched = False


def _patch_tile_drain():
    """Walrus in this container accepts only one sync-wait per InstDrain;
    split the tile-exit drain's waits across one drain instruction each."""
    global _patched
    if _patched:
        return
    _patched = True

    def _drain_and_barrier(self, tick_clock, wait_clock):
        nc = self.nc
        drain_inst = nc.sync.drain()
        wait_clock.add_sem_waits(
            drain_inst.ins, ScopedClock({None: tick_clock.global_clock})
        )
        si = drain_inst.ins.sync_info
        if si is not None and len(si.on_wait) > 1:
            waits = list(si.on_wait)
            updates = list(si.on_update)
            drain_inst.ins.sync_info = mybir.SyncInfo(
                on_wait=[waits[0]], on_update=updates
            )
            for w in waits[1:]:
                extra = nc.sync.drain()
                extra.ins.sync_info = mybir.SyncInfo(on_wait=[w], on_update=[])
        nc.all_engine_barrier()
        popped = nc._tile_sem_poison_stack.pop()
        assert popped is self._sem_poison
        # Skip Tile's exit-time sem clear + second barrier: walrus's
        # end-of-NEFF epilogue unconditionally zeroes every semaphore on
        # every engine, and nothing runs between the barrier above and
        # that epilogue. (Verified: repeated executions stay correct.)
        sems = list(self.sems.allocated().values())
        sem_nums = [s.num if hasattr(s, "num") else s for s in sems]
        nc._state.prepend_free_semaphores(sem_nums)
        for poison_set in nc._tile_sem_poison_stack:
            poison_set.update(sem_nums)

    tile.TileContext._drain_and_barrier = _drain_and_barrier


def _hybrid_halves(m3=M3):
    """Evenly spread m3 hybrid halves over half-units 2..NH-1 (the first
    two stay fp16 so cold-clock fp8 matmuls never land in the HAM ramp
    window)."""
    if m3 == 0:
        return set()
    lo, hi = 2, NH - 1
    span = hi - lo
    return {lo + (i * span) // (m3 - 1) if m3 > 1 else lo for i in range(m3)}


def _schedule(m3):
    """Per-group: (rows, half descriptors). Half descriptor: (global half
    index, hybrid?, index among the group's fp16 halves, index among the
    group's hybrid halves)."""
    hyb = _hybrid_halves(m3)
    sched = []
    hidx = 0
    for B in GROUPS:
        halves = []
        i16 = i8 = 0
        for _ in range(B // H):
            is_h = hidx in hyb
            halves.append((hidx, is_h, i16, i8))
            if is_h:
                i8 += 1
            else:
                i16 += 1
            hidx += 1
        sched.append((B, halves, i16, i8))
    return sched


_nc_cache = {}


def _build(m3=M3, nwarm=NWARM):
    key = (m3, nwarm)
    if key in _nc_cache:
        return _nc_cache[key]
    _patch_tile_drain()
    sched = _schedule(m3)

    n_lo = sum(2 * B for B, _, _, _ in sched)
    n_hi = sum(2 * H * n16 for _, _, n16, _ in sched)
    n_x8 = sum(2 * H * n8 for _, _, _, n8 in sched)

    nc = bacc.Bacc(None, target_bir_lowering=False)
    xP16 = nc.dram_tensor("xP16", [P, n_lo + n_hi], F16, kind="ExternalInput")
    xP8 = nc.dram_tensor("xP8", [P, max(1, n_x8)], F8, kind="ExternalInput")
    wP16 = nc.dram_tensor("wP16", [P, KT * BOUT], F16, kind="ExternalInput")
    wP8 = nc.dram_tensor("wP8", [P, 2 * BOUT], F8, kind="ExternalInput")
    biasP = nc.dram_tensor("biasP", [P, KT], F32, kind="ExternalInput")
    yP = nc.dram_tensor("yP", [P, NH * KT * H], F16, kind="ExternalOutput")

    with tile.TileContext(nc) as tc:
        with (
            tc.tile_pool(name="const", bufs=1) as const,
            tc.tile_pool(name="xlo", bufs=4) as xlo,
            tc.tile_pool(name="xhi", bufs=3) as xhi,
            tc.tile_pool(name="x8p", bufs=3) as x8p,
            tc.tile_pool(name="yp", bufs=3) as yp,
            tc.tile_pool(name="ps", bufs=8, space="PSUM") as psp,
        ):
            # PE warmup on memset scratch: no DMA dependency, keeps the
            # HAM clock-gate window busy while the first x DMAs land.
            # memset on the otherwise-idle gpsimd engine (vector's early
            # instruction slots come up later than gpsimd's).
            scratch = const.tile([P, 640], F16)
            nc.gpsimd.memset(scratch[:], 0.25)
            for _ in range(nwarm):
                wps = psp.tile([P, H], F32, tag="ps")
                nc.tensor.matmul(
                    wps[:], scratch[:, 512:], scratch[:, :512], start=True, stop=True
                )

            # Weights on the gpsimd queue (one-time, ~5KB/partition), split
            # so the k=0/1 half (needed by the first real matmul) lands
            # ahead of the rest. gpsimd issues them ~1.5us earlier than
            # scalar, whose stream is blocked by the activation-table load.
            w16 = const.tile([P, KT, BOUT], F16)
            wflat = w16[:].rearrange("p c o -> p (c o)")
            nc.gpsimd.dma_start(wflat[:, : 2 * BOUT], wP16[:, : 2 * BOUT])
            nc.gpsimd.dma_start(wflat[:, 2 * BOUT :], wP16[:, 2 * BOUT :])
            w8 = const.tile([P, 2, BOUT], F8)
            nc.gpsimd.dma_start(w8[:].rearrange("p j o -> p (j o)"), wP8[:])
            bt = const.tile([P, KT], F32)
            # tiny bias transfer first on sync: it absorbs the queue's
            # cold-start latency so the critical group-0 x piece behind it
            # starts moving on an already-awake queue
            nc.sync.dma_start(bt[:], biasP[:])

            off16 = 0
            off8 = 0
            row = 0
            for gi, (B, halves, n16, n8) in enumerate(sched):
                lo_t = xlo.tile([P, 2, B], F16, tag="lo")
                lo_flat = lo_t[:].rearrange("p c b -> p (c b)")
                if gi < 2:
                    # head groups: per-chunk pieces so the first matmul's
                    # data is visible as early as possible
                    for c in range(2):
                        nc.sync.dma_start(
                            lo_flat[:, c * B : (c + 1) * B],
                            xP16[:, off16 + c * B : off16 + (c + 1) * B],
                        )
                else:
                    nc.sync.dma_start(lo_flat, xP16[:, off16 : off16 + 2 * B])
                off16 += 2 * B
                if n8:
                    # x8 ahead of the hi chunks: it is small (2-3KB/part)
                    # and a late arrival stalls the group's DR matmuls
                    x8_t = x8p.tile([P, 2, H * n8], F8, tag="x8")
                    nc.sync.dma_start(
                        x8_t[:].rearrange("p j b -> p (j b)"),
                        xP8[:, off8 : off8 + 2 * H * n8],
                    )
                    off8 += 2 * H * n8
                if n16:
                    hi_t = xhi.tile([P, 2, H * n16], F16, tag="hi")
                    hi_flat = hi_t[:].rearrange("p c b -> p (c b)")
                    if gi < 2:
                        for c in range(2):
                            nc.sync.dma_start(
                                hi_flat[:, c * H * n16 : (c + 1) * H * n16],
                                xP16[:, off16 + c * H * n16 : off16 + (c + 1) * H * n16],
                            )
                    else:
                        nc.sync.dma_start(hi_flat, xP16[:, off16 : off16 + 2 * H * n16])
                    off16 += 2 * H * n16
                nh = B // H
                yt = yp.tile([P, nh, KT, H], F16, tag="yt")
                for hl, (hidx, is_h, i16, i8) in enumerate(halves):
                    bsl = slice(hl * H, (hl + 1) * H)
                    for ot in range(KT):
                        ps = psp.tile([P, H], F32, tag="ps")
                        osl = slice(ot * P, (ot + 1) * P)
                        nc.tensor.matmul(
                            ps[:], w16[:, 0, osl], lo_t[:, 0, bsl],
                            start=True, stop=False,
                        )
                        nc.tensor.matmul(
                            ps[:], w16[:, 1, osl], lo_t[:, 1, bsl],
                            start=False, stop=False,
                        )
                        if is_h:
                            nc.tensor.matmul(
                                ps[:], w8[:, :, osl],
                                x8_t[:, :, i8 * H : (i8 + 1) * H],
                                start=False, stop=True,
                                perf_mode=mybir.MatmulPerfMode.DoubleRow,
                            )
                        else:
                            nc.tensor.matmul(
                                ps[:], w16[:, 2, osl],
                                hi_t[:, 0, i16 * H : (i16 + 1) * H],
                                start=False, stop=False,
                            )
                            nc.tensor.matmul(
                                ps[:], w16[:, 3, osl],
                                hi_t[:, 1, i16 * H : (i16 + 1) * H],
                                start=False, stop=True,
                            )
                        # dual-engine drain: bias[o] is per-partition here
                        if (hidx * KT + ot) % 2 == 0:
                            nc.vector.tensor_scalar_add(
                                yt[:, hl, ot, :], ps[:], bt[:, ot : ot + 1]
                            )
                        else:
                            nc.scalar.add(yt[:, hl, ot, :], ps[:], bt[:, ot : ot + 1])

                # y out in per-half-pair pieces alternating between the
                # gpsimd and scalar queues: completion is incremental and
                # the end-of-run backlog drains at 2x queue bandwidth
                ybase = row * KT
                ydst = yP[:, ybase : ybase + nh * KT * H]
                if gi >= len(sched) - 2:
                    # final groups: per-half pieces on both queues (the
                    # scalar engine is nearly idle by now) so the y
                    # backlog drains at 2x queue bandwidth
                    for hl in range(nh):
                        q = nc.gpsimd if hl % 2 == 0 else nc.scalar
                        q.dma_start(
                            ydst[:, hl * KT * H : (hl + 1) * KT * H],
                            yt[:, hl, :, :],
                        )
                else:
                    # one DMA per group on the gpsimd queue: the scalar
                    # engine's drain cadence paces the PE, keep it clear;
                    # sync still owns the x pipeline at this point
                    nc.gpsimd.dma_start(ydst, yt[:])
                row += B
    nc.compile()
    _nc_cache[key] = nc
    return nc


def _prep_core(xg, Wg, bg, m3):
    """Host-side layout for one core: xg [BATCH, 512] f32, Wg [512, 512]
    (torch Linear layout: [out, in]), bg [512]."""
    sched = _schedule(m3)
    x16_blocks = []
    x8_blocks = []
    row = 0
    for B, halves, n16, n8 in sched:
        xT = xg[row : row + B].T  # [512k, B]
        row += B
        lo = xT[:256].reshape(2, P, B).transpose(1, 0, 2)
        x16_blocks.append(np.ascontiguousarray(lo.astype(np.float16).reshape(P, 2 * B)))
        if n16:
            cols = np.concatenate(
                [xT[256:, hl * H : (hl + 1) * H] for hl, (_, is_h, _, _) in enumerate(halves) if not is_h],
                axis=1,
            )  # [256, H*n16]
            hi = cols.reshape(2, P, H * n16).transpose(1, 0, 2)
            x16_blocks.append(np.ascontiguousarray(hi.astype(np.float16).reshape(P, -1)))
        if n8:
            cols = np.concatenate(
                [xT[256:, hl * H : (hl + 1) * H] for hl, (_, is_h, _, _) in enumerate(halves) if is_h],
                axis=1,
            )
            h8 = (cols * SX).reshape(2, P, H * n8).transpose(1, 0, 2)
            x8_blocks.append(np.ascontiguousarray(h8.astype(NPF8).reshape(P, -1)))
    xP16 = np.ascontiguousarray(np.concatenate(x16_blocks, axis=1))
    if x8_blocks:
        xP8 = np.ascontiguousarray(np.concatenate(x8_blocks, axis=1))
    else:
        xP8 = np.zeros((P, 1), NPF8)

    WT = Wg.T  # [in k, out o]
    wP16 = np.ascontiguousarray(
        WT.reshape(KT, P, BOUT).transpose(1, 0, 2).astype(np.float16).reshape(P, KT * BOUT)
    )
    wP8 = np.ascontiguousarray(
        (WT[256:] * SW).reshape(2, P, BOUT).transpose(1, 0, 2).astype(NPF8).reshape(P, 2 * BOUT)
    )
    biasP = np.ascontiguousarray(bg.reshape(KT, P).T.astype(np.float32))
    return {"xP16": xP16, "xP8": xP8, "wP16": wP16, "wP8": wP8, "biasP": biasP}


LAST_RESULT = None


def kernel(x, W, b, trace=False, m3=M3, nwarm=NWARM, trace_kwargs=None):
    global LAST_RESULT
    x = np.asarray(x, dtype=np.float32)
    W = np.asarray(W, dtype=np.float32)
    b = np.asarray(b, dtype=np.float32)

    nc = _build(m3, nwarm)

    in_maps = [
        _prep_core(x[:, g * BIN : (g + 1) * BIN], W[g], b[g], m3)
        for g in range(NCORES)
    ]

    kwargs = dict(trace_kwargs or {})
    res = run_bass_kernel_spmd(nc, in_maps, list(range(NCORES)), trace=trace, **kwargs)
    LAST_RESULT = res

    out = np.empty((BATCH, NB * BOUT), dtype=np.float32)
    for g in range(NCORES):
        # yP columns per half-unit: [KT(ot), H(c)]; rows r = 512*hidx + c
        arr = res.results[g]["yP"].reshape(P, NH, KT, H)
        blk = arr.transpose(1, 3, 2, 0).reshape(BATCH, BOUT)
        out[:, g * BOUT : (g + 1) * BOUT] = blk.astype(np.float32)
    return out
